# revision 8
# baseline (speedup 1.0000x reference)
"""Trainium2 Bass kernel for nn_AddDropMRR (add-drop microring resonator).

Math: both outputs are magnitudes of complex-linear maps of the two real
inputs, through = |alpha_w x + beta_w a|, drop = |ad_w x + bd_w a|, so

  through^2 = A x^2 + B xa + C a^2      A=|alpha|^2, B=2Re(alpha conj(beta)),
  drop^2    = D x^2 + E xa + F a^2      C=|beta|^2  (etc. for drop)

with all six coefficients per-wavelength functions of `wavelengths` and the
scalar params -> computed on HOST (complex128) and shipped once as f16
diagonal matmul blocks. A and C (D and F) are inflated by (1+2^-9) to keep
the quadratic form non-negative under f16 product rounding (the form is PSD:
discriminant = Im(alpha conj(beta))^2 >= 0), and the ACT sqrt adds a tiny
positive bias to absorb f32 accumulation dips.

Device graph per chunk (128 wavelengths x 2048 batch), software-pipelined:

  s0 DMA(sync q):  load x, a chunk                  (~3.2us/chunk shared-DMA)
  s1 DVE:          xa = x*a, xx = x*x (in place)    (~2.3us)
      Pool:        aa = a*a (in place)              (~1.7us)
  s2 PE:           PSUM_T[h] = dgA@xx + dgB@xa + dgC@aa   (per 1024-half,
                   PSUM_D[h] = dgD@xx + dgE@xa + dgF@aa    512-col groups)
  s3 ACT:          o1 = sqrt(PSUM_T + 1e-5), o2 = sqrt(PSUM_D + 1e-7) -> f16
                   o1 DMA on scalar q, o2 DMA on vector q

This cuts DVE busy from ~55us (baseline u/v rotation form: 7 DVE ops/chunk)
to ~18us (2 ops/chunk); PE does 6 diag sweeps/chunk (~46us) and the shared
DMA device (~50us for the 16MiB/core of fp16 I/O) becomes the bound.
Sharding: wavelength dim split 8 ways across cores (fully elementwise);
host transposes so wavelength lies on SBUF partitions. Coef tables load
outside the timing loop.
"""
import numpy as np

B = 2048           # batch
W = 8192           # wavelengths
NCORES = 8
WSH = W // NCORES  # 1024 wavelengths per core
P = 128            # SBUF partitions
NCHUNK = WSH // P  # 8 chunks per core
NCOEF = 6          # A, B, C, D, E, F
N_EFF = 2.4
CIRC = 2.0 * np.pi * 1e-05
DLT = 2.0 ** -9    # PSD inflation of A, C, D, F
MODE = "q"         # quadratic-form mode (only mode)
HB = B // 2        # psum half width


def _host_prep(wavelengths, coupling_1, coupling_2, phi_1, phi_2, phi_ring,
               alpha):
    """Six per-wavelength quadratic-form coefficients (complex128 host)."""
    c1 = float(np.asarray(coupling_1).reshape(-1)[0])
    c2 = float(np.asarray(coupling_2).reshape(-1)[0])
    p1 = float(np.asarray(phi_1).reshape(-1)[0])
    pr = float(np.asarray(phi_ring).reshape(-1)[0])
    al = float(np.asarray(alpha).reshape(-1)[0])
    k1c = float(np.clip(c1, 0.01, 0.99))
    k2c = float(np.clip(c2, 0.01, 0.99))
    t1 = float(np.sqrt(1.0 - k1c * k1c))
    t2 = float(np.sqrt(1.0 - k2c * k2c))
    s1 = float(np.sqrt(c1))      # unclamped, as in reference
    s = float(np.sqrt(c2))       # unclamped
    kappa = float(al * np.sqrt(1.0 - c1 * c1) * np.sqrt(1.0 - c2 * c2))

    # phi in f32 exactly as the reference computes it, then f64 trig
    wl = np.asarray(wavelengths, np.float32)
    phi32 = (np.float32(2.0 * np.pi * N_EFF) / wl) * np.float32(CIRC) \
        + np.float32(pr)
    phi = phi32.astype(np.float64)

    den = 1.0 - kappa * np.exp(1j * phi)
    ring = 1j * k1c * al * np.exp(1j * (phi + p1))     # ring one-pass factor
    alpha_t = t1 + t2 * s1 * ring / den                # through: x coef
    beta_t = (t2 * s1 * s) / den                       # through: a coef
    ad = k2c * ring                                    # |j e^{j phi2}| = 1
    bd = (k2c * s) * np.ones_like(phi)

    A = (np.abs(alpha_t) ** 2) * (1.0 + DLT)
    Bv = 2.0 * np.real(alpha_t * np.conj(beta_t))
    C = (np.abs(beta_t) ** 2) * (1.0 + DLT)
    D = (np.abs(ad) ** 2) * (1.0 + DLT)
    E = 2.0 * np.real(ad * np.conj(bd))
    F = (np.abs(bd) ** 2) * (1.0 + DLT)

    coefs = np.stack([A, Bv, C, D, E, F]).astype(np.float32)   # [NCOEF, W]
    return coefs, dict(s=s, k2c=k2c)


def _build_graph(k2c, loop_n=1, nchunk=NCHUNK, bufs=8, taper=1, **_ignored):
    """SPMD per-core graph; see module docstring. loop_n>1 wraps the body
    in an on-device For_i loop for steady-state timing; the coefficient
    table loads stay OUTSIDE the loop."""
    import concourse.tile as tile
    from concourse import bacc, mybir, bass

    f16 = mybir.dt.float16
    f32 = mybir.dt.float32
    AF = mybir.ActivationFunctionType
    ndiag = NCOEF * nchunk

    wsh = nchunk * P
    nc = bacc.Bacc("TRN2", target_bir_lowering=False, debug=False,
                   num_devices=NCORES)
    x_ext = nc.declare_dram_parameter("x_t", [wsh, B], f16, isOutput=False)
    a_ext = nc.declare_dram_parameter("a_t", [wsh, B], f16, isOutput=False)
    dg_ext = nc.declare_dram_parameter("dg_t", [P, ndiag * P], f16,
                                       isOutput=False)
    o1_ext = nc.declare_dram_parameter("o1_t", [wsh, B], f16, isOutput=True)
    o2_ext = nc.declare_dram_parameter("o2_t", [wsh, B], f16, isOutput=True)

    with tile.TileContext(nc) as tc:
        with tc.tile_pool(name="cst", bufs=1) as cst, \
             tc.tile_pool(name="mio", bufs=bufs) as mio, \
             tc.tile_pool(name="mout", bufs=4) as mout, \
             tc.tile_pool(name="psum", bufs=2,
                          space=bass.MemorySpace.PSUM) as psum:

            dg = cst.tile([P, ndiag * P], f16, tag="dg", name="dg")
            nc.sync.dma_start(dg[:], dg_ext[:])
            b1 = cst.tile([P, 1], f32, tag="b1", name="b1")
            nc.vector.memset(b1[:], 1e-5)
            b2 = cst.tile([P, 1], f32, tag="b2", name="b2")
            nc.vector.memset(b2[:], 1e-7)

            def DG(k, c):  # diag block of coef k, chunk c
                b = k * nchunk + c
                return dg[:, b * P:(b + 1) * P]

            def body(_iv=None):
                st = {}

                def s0(vc):
                    v_id, c, lo, hi = vc
                    rs = slice(c * P, (c + 1) * P)
                    cs = slice(lo, hi)
                    xt = mio.tile([P, B], f16, tag="xt", name="xt")
                    nc.sync.dma_start(xt[:, cs], x_ext[rs, cs])
                    at = mio.tile([P, B], f16, tag="at", name="at")
                    nc.sync.dma_start(at[:, cs], a_ext[rs, cs])
                    st[v_id] = dict(xt=xt, at=at)

                def s1(vc):
                    v_id, c, lo, hi = vc
                    cs = slice(lo, hi)
                    d = st[v_id]
                    xt, at = d["xt"], d["at"]
                    xa = mio.tile([P, B], f16, tag="xa", name="xa")
                    nc.vector.tensor_mul(xa[:, cs], xt[:, cs], at[:, cs])
                    nc.vector.tensor_mul(xt[:, cs], xt[:, cs], xt[:, cs])
                    nc.gpsimd.tensor_mul(at[:, cs], at[:, cs], at[:, cs])
                    d["xa"] = xa

                def s2(vc):
                    v_id, c, lo, hi = vc
                    d = st[v_id]
                    xx, xa, aa = d["xt"], d["xa"], d["at"]
                    groups = [(goff, min(HB, hi - goff))
                              for goff in range(lo, hi, HB)]
                    tps, dps = [], []
                    d["tps"], d["dps"], d["groups"] = tps, dps, groups
                    mm = nc.tensor.matmul

                    # group-major: finish each PSUM accumulator quickly so
                    # its bank recycles while the next group accumulates
                    def one(dst, k0, goff, gw):
                        for ki, src in ((0, xx), (1, xa), (2, aa)):
                            dgb = DG(k0 + ki, c)
                            for j in range(0, gw, 512):
                                w = min(512, gw - j)
                                mm(dst[:, j:j + w], dgb,
                                   src[:, goff + j:goff + j + w],
                                   start=(ki == 0), stop=(ki == 2))

                    for goff, gw in groups:
                        tp = psum.tile([P, HB], f32, tag="tp", name="tp")
                        tps.append(tp)
                        one(tp, 0, goff, gw)
                        dp = psum.tile([P, HB], f32, tag="dp", name="dp")
                        dps.append(dp)
                        one(dp, 3, goff, gw)

                def s3(vc):
                    v_id, c, lo, hi = vc
                    d = st.pop(v_id)
                    rs = slice(c * P, (c + 1) * P)
                    cs = slice(lo, hi)
                    o1t = mout.tile([P, B], f16, tag="o1t", name="o1t")
                    o2t = mout.tile([P, B], f16, tag="o2t", name="o2t")
                    for h, (goff, gw) in enumerate(d["groups"]):
                        hs = slice(goff, goff + gw)
                        nc.scalar.activation(o1t[:, hs], d["tps"][h][:, 0:gw],
                                             AF.Sqrt, bias=b1[:])
                        nc.scalar.activation(o2t[:, hs], d["dps"][h][:, 0:gw],
                                             AF.Sqrt, bias=b2[:])
                    nc.scalar.dma_start(o1_ext[rs, cs], o1t[:, cs])
                    nc.gpsimd.dma_start(o2_ext[rs, cs], o2t[:, cs])

                stages = [s0, s1, s2, s3]
                nstg = len(stages)
                # first/last chunks split into column halves so the pipeline
                # fills fast and drains with a short tail
                spans = []
                for c in range(nchunk):
                    if taper and c in (0, nchunk - 1):
                        spans += [(c, 0, HB), (c, HB, B)]
                    else:
                        spans.append((c, 0, B))
                vchunks = [(i, c, lo, hi)
                           for i, (c, lo, hi) in enumerate(spans)]
                nv = len(vchunks)
                for t in range(nv + nstg - 1):
                    for s in range(nstg - 1, -1, -1):
                        i = t - s
                        if 0 <= i < nv:
                            stages[s](vchunks[i])

            if loop_n > 1:
                with tc.For_i(0, loop_n, 1):
                    body()
            else:
                body()

    nc.compile()
    return nc


def _shard_inputs(input_signal, add_signal, coefs, s, vecs=None, mode=MODE):
    x = np.asarray(input_signal, dtype=np.float32).astype(np.float16)
    a = np.asarray(add_signal, dtype=np.float32).astype(np.float16)
    in_maps = []
    for i in range(NCORES):
        sl = slice(i * WSH, (i + 1) * WSH)
        dgm = np.zeros((P, NCOEF * NCHUNK * P), np.float16)
        csh = coefs[:, sl].reshape(NCOEF, NCHUNK, P)
        for k in range(NCOEF):
            for c in range(NCHUNK):
                b = k * NCHUNK + c
                dgm[:, b * P:(b + 1) * P] = np.diag(
                    csh[k, c].astype(np.float16))
        in_maps.append({
            "x_t": np.ascontiguousarray(x[:, sl].T),
            "a_t": np.ascontiguousarray(a[:, sl].T),
            "dg_t": dgm,
        })
    return in_maps


def _gather_outputs(results):
    through = np.empty((B, W), np.float32)
    drop = np.empty((B, W), np.float32)
    for i in range(NCORES):
        sl = slice(i * WSH, (i + 1) * WSH)
        through[:, sl] = results[i]["o1_t"].T.astype(np.float32)
        drop[:, sl] = results[i]["o2_t"].T.astype(np.float32)
    return through, drop


def kernel(input_signal, add_signal, wavelengths, coupling_1, coupling_2,
           phi_1, phi_2, phi_ring, alpha):
    from concourse.bass_utils import run_bass_kernel_spmd

    coefs, sc = _host_prep(wavelengths, coupling_1, coupling_2, phi_1, phi_2,
                           phi_ring, alpha)
    nc = _build_graph(sc["k2c"])
    in_maps = _shard_inputs(input_signal, add_signal, coefs, sc["s"])
    res = run_bass_kernel_spmd(nc, in_maps, core_ids=list(range(NCORES)))
    return _gather_outputs(res.results)


# revision 10
# speedup vs baseline: 1.0460x; 1.0460x over previous
"""Trainium2 Bass kernel for nn_AddDropMRR (add-drop microring resonator).

Math: both outputs are magnitudes of complex-linear maps of the two real
inputs, through = |alpha_w x + beta_w a|, drop = |ad_w x + bd_w a|, so

  through^2 = A x^2 + B xa + C a^2      A=|alpha|^2, B=2Re(alpha conj(beta)),
  drop^2    = D x^2 + E xa + F a^2      C=|beta|^2  (etc. for drop)

with all six coefficients per-wavelength functions of `wavelengths` and the
scalar params -> computed on HOST (complex128) and shipped once as f16
diagonal matmul blocks. A and C (D and F) are inflated by (1+2^-9) to keep
the quadratic form non-negative under f16 product rounding (the form is PSD:
discriminant = Im(alpha conj(beta))^2 >= 0), and the ACT sqrt adds a tiny
positive bias to absorb f32 accumulation dips.

Device graph per chunk (128 wavelengths x 2048 batch), software-pipelined:

  s0 DMA(sync q):  load x, a chunk                  (~3.2us/chunk shared-DMA)
  s1 DVE:          xa = x*a, xx = x*x (in place)    (~2.3us)
      Pool:        aa = a*a (in place)              (~1.7us)
  s2 PE:           PSUM_T[h] = dgA@xx + dgB@xa + dgC@aa   (per 1024-half,
                   PSUM_D[h] = dgD@xx + dgE@xa + dgF@aa    512-col groups)
  s3 ACT:          o1 = sqrt(PSUM_T + 1e-5), o2 = sqrt(PSUM_D + 1e-7) -> f16
                   o1 DMA on scalar q, o2 DMA on vector q

This cuts DVE busy from ~55us (baseline u/v rotation form: 7 DVE ops/chunk)
to ~18us (2 ops/chunk); PE does 6 diag sweeps/chunk (~46us) and the shared
DMA device (~50us for the 16MiB/core of fp16 I/O) becomes the bound.
Sharding: wavelength dim split 8 ways across cores (fully elementwise);
host transposes so wavelength lies on SBUF partitions. Coef tables load
outside the timing loop.
"""
import numpy as np

B = 2048           # batch
W = 8192           # wavelengths
NCORES = 8
WSH = W // NCORES  # 1024 wavelengths per core
P = 128            # SBUF partitions
NCHUNK = WSH // P  # 8 chunks per core
NCOEF = 6          # A, B, C, D, E, F
N_EFF = 2.4
CIRC = 2.0 * np.pi * 1e-05
DLT = 2.0 ** -9    # PSD inflation of A, C, D, F
MODE = "q"         # quadratic-form mode (only mode)
HB = B // 2        # psum half width


def _host_prep(wavelengths, coupling_1, coupling_2, phi_1, phi_2, phi_ring,
               alpha):
    """Six per-wavelength quadratic-form coefficients (complex128 host)."""
    c1 = float(np.asarray(coupling_1).reshape(-1)[0])
    c2 = float(np.asarray(coupling_2).reshape(-1)[0])
    p1 = float(np.asarray(phi_1).reshape(-1)[0])
    pr = float(np.asarray(phi_ring).reshape(-1)[0])
    al = float(np.asarray(alpha).reshape(-1)[0])
    k1c = float(np.clip(c1, 0.01, 0.99))
    k2c = float(np.clip(c2, 0.01, 0.99))
    t1 = float(np.sqrt(1.0 - k1c * k1c))
    t2 = float(np.sqrt(1.0 - k2c * k2c))
    s1 = float(np.sqrt(c1))      # unclamped, as in reference
    s = float(np.sqrt(c2))       # unclamped
    kappa = float(al * np.sqrt(1.0 - c1 * c1) * np.sqrt(1.0 - c2 * c2))

    # phi in f32 exactly as the reference computes it, then f64 trig
    wl = np.asarray(wavelengths, np.float32)
    phi32 = (np.float32(2.0 * np.pi * N_EFF) / wl) * np.float32(CIRC) \
        + np.float32(pr)
    phi = phi32.astype(np.float64)

    den = 1.0 - kappa * np.exp(1j * phi)
    ring = 1j * k1c * al * np.exp(1j * (phi + p1))     # ring one-pass factor
    alpha_t = t1 + t2 * s1 * ring / den                # through: x coef
    beta_t = (t2 * s1 * s) / den                       # through: a coef
    ad = k2c * ring                                    # |j e^{j phi2}| = 1
    bd = (k2c * s) * np.ones_like(phi)

    A = (np.abs(alpha_t) ** 2) * (1.0 + DLT)
    Bv = 2.0 * np.real(alpha_t * np.conj(beta_t))
    C = (np.abs(beta_t) ** 2) * (1.0 + DLT)
    D = (np.abs(ad) ** 2) * (1.0 + DLT)
    E = 2.0 * np.real(ad * np.conj(bd))
    F = (np.abs(bd) ** 2) * (1.0 + DLT)

    coefs = np.stack([A, Bv, C, D, E, F]).astype(np.float32)   # [NCOEF, W]
    return coefs, dict(s=s, k2c=k2c)


def _build_graph(k2c, loop_n=1, nchunk=NCHUNK, bufs=8, taper=1, **_ignored):
    """SPMD per-core graph; see module docstring. loop_n>1 wraps the body
    in an on-device For_i loop for steady-state timing; the coefficient
    table loads stay OUTSIDE the loop."""
    import concourse.tile as tile
    from concourse import bacc, mybir, bass

    f16 = mybir.dt.float16
    f32 = mybir.dt.float32
    AF = mybir.ActivationFunctionType
    ndiag = NCOEF * nchunk

    wsh = nchunk * P
    nc = bacc.Bacc("TRN2", target_bir_lowering=False, debug=False,
                   num_devices=NCORES)
    x_ext = nc.declare_dram_parameter("x_t", [wsh, B], f16, isOutput=False)
    a_ext = nc.declare_dram_parameter("a_t", [wsh, B], f16, isOutput=False)
    dg_ext = nc.declare_dram_parameter("dg_t", [P, ndiag * P], f16,
                                       isOutput=False)
    o1_ext = nc.declare_dram_parameter("o1_t", [wsh, B], f16, isOutput=True)
    o2_ext = nc.declare_dram_parameter("o2_t", [wsh, B], f16, isOutput=True)

    with tile.TileContext(nc) as tc:
        with tc.tile_pool(name="cst", bufs=1) as cst, \
             tc.tile_pool(name="mio", bufs=bufs) as mio, \
             tc.tile_pool(name="mout", bufs=4) as mout, \
             tc.tile_pool(name="psum", bufs=2,
                          space=bass.MemorySpace.PSUM) as psum:

            dg = cst.tile([P, ndiag * P], f16, tag="dg", name="dg")
            nc.sync.dma_start(dg[:], dg_ext[:])
            b1 = cst.tile([P, 1], f32, tag="b1", name="b1")
            nc.vector.memset(b1[:], 1e-5)
            b2 = cst.tile([P, 1], f32, tag="b2", name="b2")
            nc.vector.memset(b2[:], 1e-7)

            def DG(k, c):  # diag block of coef k, chunk c
                b = k * nchunk + c
                return dg[:, b * P:(b + 1) * P]

            def body(_iv=None):
                st = {}

                def s0(vc):
                    v_id, c, lo, hi = vc
                    rs = slice(c * P, (c + 1) * P)
                    cs = slice(lo, hi)
                    xt = mio.tile([P, B], f16, tag="xt", name="xt")
                    nc.sync.dma_start(xt[:, cs], x_ext[rs, cs])
                    at = mio.tile([P, B], f16, tag="at", name="at")
                    nc.sync.dma_start(at[:, cs], a_ext[rs, cs])
                    st[v_id] = dict(xt=xt, at=at)

                def s1(vc):
                    v_id, c, lo, hi = vc
                    cs = slice(lo, hi)
                    d = st[v_id]
                    xt, at = d["xt"], d["at"]
                    xa = mio.tile([P, B], f16, tag="xa", name="xa")
                    nc.vector.tensor_mul(xa[:, cs], xt[:, cs], at[:, cs])
                    nc.vector.tensor_mul(xt[:, cs], xt[:, cs], xt[:, cs])
                    nc.vector.tensor_mul(at[:, cs], at[:, cs], at[:, cs])
                    d["xa"] = xa

                def s2(vc):
                    v_id, c, lo, hi = vc
                    d = st[v_id]
                    xx, xa, aa = d["xt"], d["xa"], d["at"]
                    groups = [(goff, min(HB, hi - goff))
                              for goff in range(lo, hi, HB)]
                    tps, dps = [], []
                    d["tps"], d["dps"], d["groups"] = tps, dps, groups
                    mm = nc.tensor.matmul

                    # group-major: finish each PSUM accumulator quickly so
                    # its bank recycles while the next group accumulates
                    def one(dst, k0, goff, gw):
                        for ki, src in ((0, xx), (1, xa), (2, aa)):
                            dgb = DG(k0 + ki, c)
                            for j in range(0, gw, 512):
                                w = min(512, gw - j)
                                mm(dst[:, j:j + w], dgb,
                                   src[:, goff + j:goff + j + w],
                                   start=(ki == 0), stop=(ki == 2))

                    for goff, gw in groups:
                        tp = psum.tile([P, HB], f32, tag="tp", name="tp")
                        tps.append(tp)
                        one(tp, 0, goff, gw)
                        dp = psum.tile([P, HB], f32, tag="dp", name="dp")
                        dps.append(dp)
                        one(dp, 3, goff, gw)

                def s3(vc):
                    v_id, c, lo, hi = vc
                    d = st.pop(v_id)
                    rs = slice(c * P, (c + 1) * P)
                    cs = slice(lo, hi)
                    o1t = mout.tile([P, B], f16, tag="o1t", name="o1t")
                    o2t = mout.tile([P, B], f16, tag="o2t", name="o2t")
                    for h, (goff, gw) in enumerate(d["groups"]):
                        hs = slice(goff, goff + gw)
                        nc.scalar.activation(o1t[:, hs], d["tps"][h][:, 0:gw],
                                             AF.Sqrt, bias=b1[:])
                        nc.scalar.activation(o2t[:, hs], d["dps"][h][:, 0:gw],
                                             AF.Sqrt, bias=b2[:])
                    nc.scalar.dma_start(o1_ext[rs, cs], o1t[:, cs])
                    nc.scalar.dma_start(o2_ext[rs, cs], o2t[:, cs])

                stages = [s0, s1, s2, s3]
                nstg = len(stages)
                # first/last chunks split into column halves so the pipeline
                # fills fast and drains with a short tail
                spans = []
                for c in range(nchunk):
                    if taper and c in (0, nchunk - 1):
                        spans += [(c, 0, HB), (c, HB, B)]
                    else:
                        spans.append((c, 0, B))
                vchunks = [(i, c, lo, hi)
                           for i, (c, lo, hi) in enumerate(spans)]
                nv = len(vchunks)
                for t in range(nv + nstg - 1):
                    for s in range(nstg - 1, -1, -1):
                        i = t - s
                        if 0 <= i < nv:
                            stages[s](vchunks[i])

            if loop_n > 1:
                with tc.For_i(0, loop_n, 1):
                    body()
            else:
                body()

    nc.compile()
    return nc


def _shard_inputs(input_signal, add_signal, coefs, s, vecs=None, mode=MODE):
    x = np.asarray(input_signal, dtype=np.float32).astype(np.float16)
    a = np.asarray(add_signal, dtype=np.float32).astype(np.float16)
    in_maps = []
    for i in range(NCORES):
        sl = slice(i * WSH, (i + 1) * WSH)
        dgm = np.zeros((P, NCOEF * NCHUNK * P), np.float16)
        csh = coefs[:, sl].reshape(NCOEF, NCHUNK, P)
        for k in range(NCOEF):
            for c in range(NCHUNK):
                b = k * NCHUNK + c
                dgm[:, b * P:(b + 1) * P] = np.diag(
                    csh[k, c].astype(np.float16))
        in_maps.append({
            "x_t": np.ascontiguousarray(x[:, sl].T),
            "a_t": np.ascontiguousarray(a[:, sl].T),
            "dg_t": dgm,
        })
    return in_maps


def _gather_outputs(results):
    through = np.empty((B, W), np.float32)
    drop = np.empty((B, W), np.float32)
    for i in range(NCORES):
        sl = slice(i * WSH, (i + 1) * WSH)
        through[:, sl] = results[i]["o1_t"].T.astype(np.float32)
        drop[:, sl] = results[i]["o2_t"].T.astype(np.float32)
    return through, drop


def kernel(input_signal, add_signal, wavelengths, coupling_1, coupling_2,
           phi_1, phi_2, phi_ring, alpha):
    from concourse.bass_utils import run_bass_kernel_spmd

    coefs, sc = _host_prep(wavelengths, coupling_1, coupling_2, phi_1, phi_2,
                           phi_ring, alpha)
    nc = _build_graph(sc["k2c"])
    in_maps = _shard_inputs(input_signal, add_signal, coefs, sc["s"])
    res = run_bass_kernel_spmd(nc, in_maps, core_ids=list(range(NCORES)))
    return _gather_outputs(res.results)
